# revision 39
# baseline (speedup 1.0000x reference)
"""MoE layer (top-2 of 8 experts + shared expert) as a Bass/Tile kernel on 8 TRN2 cores.

Strategy (expert parallelism, per the sharding hint):
  - Host computes the tiny gating network (softmax -> top-2 -> renormalize) and
    builds the all-to-all token dispatch: core e receives the tokens routed to
    expert e (padded to a small fixed capacity C), pre-transposed to [d_model, C].
  - Core e runs expert e's FFN on its tokens (silu(x@W1 * x@W3) @ W2), scales
    each output token by its gate weight, and also runs a 512-token slice of the
    shared expert (token-parallel across the 8 cores).
  - Host scatter-adds the two expert contributions per token and the shared
    output back into the full [T, d] result.

All heavy FLOPs (expert FFNs + shared FFN) run on device; the host only does
O(T*E) gating math and O(T*d) data movement.

Device schedule: the shared expert runs FIRST (its weights stream in small
chunks, so the PE starts within ~2us), while the expert-path weights prefetch
underneath it; the expert path then runs entirely from SBUF-resident weights.
"""

import os
import sys

for _p in ("/opt/trn_rl_repo",):
    if _p not in sys.path and os.path.isdir(_p):
        sys.path.insert(0, _p)

import numpy as np
import ml_dtypes

import concourse.bass as bass
import concourse.mybir as mybir
import concourse.tile as tile
from concourse import bacc
from concourse.bass_utils import run_bass_kernel_spmd


def install_ntff_hook():
    """This image's antenv lacks axon_hooks, which run_bass_kernel_spmd imports
    unconditionally when tracing; provide it and register the ctypes NTFF
    profile hook so trace=True (or BASS_TRACE=1) works."""
    import types

    try:
        import antenv.axon_hooks  # noqa: F401
        return
    except ImportError:
        pass
    mod = types.ModuleType("antenv.axon_hooks")
    _hook = [None]
    mod.set_axon_ntff_profile_hook = lambda h: _hook.__setitem__(0, h)
    mod.get_axon_ntff_profile_hook = lambda: _hook[0]
    sys.modules["antenv.axon_hooks"] = mod
    try:
        import antenv

        antenv.axon_hooks = mod
    except ImportError:
        pass
    try:
        from trn_agent_boot.trn_boot import _ntff_profile_via_ctypes

        mod.set_axon_ntff_profile_hook(
            _ntff_profile_via_ctypes("/opt/axon/libaxon_pjrt.so")
        )
    except Exception:
        pass


install_ntff_hook()

P = 128
D = 1024          # d_model
F = 2048          # d_ff per expert
FS = 4096         # shared expert hidden
E = 8             # experts == cores
TOPK = 2
TS = 512          # shared-expert tokens per core (T / 8)
T = 4096

f32 = mybir.dt.float32
bf16 = mybir.dt.bfloat16

MM_CFG = os.environ.get("MOE_MM_CFG", "bf16")

_COMPILED: dict = {}


def _np_mm_dtype(cfg):
    return ml_dtypes.bfloat16 if cfg == "bf16" else np.float32


def _chunks(C):
    """Split C token columns into matmul-N chunks of <=512."""
    out = []
    s = 0
    while s < C:
        w = min(512, C - s)
        out.append((s, w))
        s += w
    return out


def build_program(C: int, cfg: str):
    """Build the per-core Bass program for expert-token capacity C."""
    assert cfg == "bf16"
    assert C % 8 == 0
    sdt = bf16

    nc = bacc.Bacc("TRN2", target_bir_lowering=False, debug=False, num_devices=E)

    # ---- per-core inputs ----
    # All inputs arrive host-pretiled in the exact SBUF layout (partition
    # dim first) so every DMA is one contiguous per-partition segment.
    DT = D // P    # 8
    FT = F // P    # 16
    FST = FS // P  # 32
    WG = 2 * P     # ws1/ws3 streaming group width

    xgT = nc.dram_tensor("xgT", [P, DT, C], sdt, kind="ExternalInput")
    gw = nc.dram_tensor("gw", [1, C], f32, kind="ExternalInput")
    w1 = nc.dram_tensor("w1", [P, DT, F], sdt, kind="ExternalInput")
    w3 = nc.dram_tensor("w3", [P, DT, F], sdt, kind="ExternalInput")
    w2 = nc.dram_tensor("w2", [P, FT, D], sdt, kind="ExternalInput")
    b1 = nc.dram_tensor("b1", [P, F // P], f32, kind="ExternalInput")
    b3 = nc.dram_tensor("b3", [P, F // P], f32, kind="ExternalInput")
    b2 = nc.dram_tensor("b2", [P, D // P], f32, kind="ExternalInput")
    xsT = nc.dram_tensor("xsT", [P, DT, TS], sdt, kind="ExternalInput")
    ws1 = nc.dram_tensor("ws1", [FS // WG, P, DT, WG], sdt, kind="ExternalInput")
    ws3 = nc.dram_tensor("ws3", [FS // WG, P, DT, WG], sdt, kind="ExternalInput")
    ws2 = nc.dram_tensor("ws2", [FS, D], sdt, kind="ExternalInput")
    bs1 = nc.dram_tensor("bs1", [P, FS // P], f32, kind="ExternalInput")
    bs3 = nc.dram_tensor("bs3", [P, FS // P], f32, kind="ExternalInput")
    bs2 = nc.dram_tensor("bs2", [P, D // P], f32, kind="ExternalInput")

    # ---- per-core outputs ----
    yT = nc.dram_tensor("yT", [D, C], f32, kind="ExternalOutput")
    ysT = nc.dram_tensor("ysT", [D, TS], f32, kind="ExternalOutput")

    CH = _chunks(C)

    with tile.TileContext(nc) as tc:
        with (
            tc.tile_pool(name="consts", bufs=1) as consts,
            tc.tile_pool(name="xg", bufs=1) as xgp,
            tc.tile_pool(name="wres", bufs=1) as wres,
            tc.tile_pool(name="wstream", bufs=3) as wstream,
            tc.tile_pool(name="w2stream", bufs=4) as w2stream,
            tc.tile_pool(name="abuf", bufs=1) as abufp,
            tc.tile_pool(name="htmp", bufs=3) as htmp,
            tc.tile_pool(name="ytmp", bufs=2) as ytmp,
            tc.tile_pool(name="ps", bufs=4, space="PSUM") as psp,
        ):
            # ---------- shared-expert inputs first (PE starts on these) ----------
            # split across DMA queues so the first matmul can start early
            # PE clock warmup: the HAM gate holds the PE at 1.2GHz until it has
            # been busy ~3.4us. Run dummy matmuls on zeroed scratch during the
            # initial input-DMA wait so the real matmuls start at 2.4GHz.
            scratch = consts.tile([P, 2 * P], sdt, tag="warmup")
            nc.vector.memset(scratch[:], 0)
            pwarm = psp.tile([P, TS], f32, tag="ph1", name="pwarm")
            N_WARM = 28
            for i in range(N_WARM):
                nc.tensor.matmul(
                    pwarm[:, : 2 * P],
                    lhsT=scratch[:, :P],
                    rhs=scratch[:],
                    start=(i == 0),
                    stop=(i == N_WARM - 1),
                )

            # issue the critical first inputs on BOTH the sync and gpsimd DMA
            # paths so per-dma first-byte latencies overlap; the first matmul
            # needs ws1 group 0 + xs d-block 0, so those issue first
            ws1_g0 = wstream.tile([P, DT, WG], sdt, tag="ws1g")
            nc.sync.dma_start(ws1_g0[:], ws1[0])
            ws3_g0 = wstream.tile([P, DT, WG], sdt, tag="ws3g")
            nc.gpsimd.dma_start(ws3_g0[:], ws3[0])
            xs_sb = xgp.tile([P, DT, TS], sdt, tag="xs")
            for i, d in enumerate(range(0, DT, 2)):
                eng = nc.sync if i % 2 == 0 else nc.gpsimd
                eng.dma_start(xs_sb[:, d : d + 2, :], xsT[:, d : d + 2, :])
            bs1_sb = consts.tile([P, FST], f32)
            nc.gpsimd.dma_start(bs1_sb[:], bs1[:])
            bs3_sb = consts.tile([P, FST], f32)
            nc.gpsimd.dma_start(bs3_sb[:], bs3[:])
            bs2_sb = consts.tile([P, DT], f32)
            nc.gpsimd.dma_start(bs2_sb[:], bs2[:])

            # a-buffer: holds a_shared [P, FST, 512] during the shared stage,
            # then two rotating [P, FT, 512] slabs for the expert chunks.
            ab = abufp.tile([P, FST * 512], sdt, tag="abuf")
            a_shared = ab[:, : FST * 512].rearrange("p (f n) -> p f n", f=FST)

            def a_expert(n):
                off = (n % 2) * (FT * 512)
                return ab[:, off : off + FT * 512].rearrange("p (f n) -> p f n", f=FT)

            # ---------- shared expert: h1s/h3s -> a_shared ----------
            for g in range(FST // 2):
                if g == 0:
                    ws1_g, ws3_g = ws1_g0, ws3_g0
                else:
                    ws1_g = wstream.tile([P, DT, WG], sdt, tag="ws1g")
                    nc.sync.dma_start(ws1_g[:], ws1[g])
                    ws3_g = wstream.tile([P, DT, WG], sdt, tag="ws3g")
                    nc.sync.dma_start(ws3_g[:], ws3[g])
                for sub in range(2):
                    fs = g * 2 + sub
                    scols = slice(sub * P, (sub + 1) * P)
                    ph1 = psp.tile([P, TS], f32, tag="ph1")
                    ph3 = psp.tile([P, TS], f32, tag="ph3")
                    for d in range(DT):
                        nc.tensor.matmul(
                            ph1[:],
                            lhsT=ws1_g[:, d, scols],
                            rhs=xs_sb[:, d, :],
                            start=(d == 0),
                            stop=(d == DT - 1),
                        )
                    for d in range(DT):
                        nc.tensor.matmul(
                            ph3[:],
                            lhsT=ws3_g[:, d, scols],
                            rhs=xs_sb[:, d, :],
                            start=(d == 0),
                            stop=(d == DT - 1),
                        )
                    h1 = htmp.tile([P, TS], f32, tag="h1")
                    nc.vector.tensor_scalar_add(h1[:], ph1[:], bs1_sb[:, fs : fs + 1])
                    prod = htmp.tile([P, TS], f32, tag="prod")
                    nc.vector.scalar_tensor_tensor(
                        prod[:],
                        in0=ph3[:],
                        scalar=bs3_sb[:, fs : fs + 1],
                        in1=h1[:],
                        op0=mybir.AluOpType.add,
                        op1=mybir.AluOpType.mult,
                    )
                    nc.scalar.activation(
                        a_shared[:, fs, :], prod[:], mybir.ActivationFunctionType.Silu
                    )

            # ---------- shared expert: ys = a_shared @ Ws2 ----------
            # single pass over Ws2 (streamed once) accumulating all 8 d-tiles
            # in 8 PSUM banks at once
            pys = [
                psp.tile([P, TS], f32, tag=("ph1" if i < 4 else "ph3"), name=f"pys_{i}")
                for i in range(DT)
            ]
            for fs in range(FST):
                ws2_b = w2stream.tile([P, D], sdt, tag="ws2b")
                nc.sync.dma_start(
                    ws2_b[:], ws2[:].rearrange("(o p) d -> p o d", p=P)[:, fs, :]
                )
                for d in range(DT):
                    nc.tensor.matmul(
                        pys[d][:],
                        lhsT=ws2_b[:, d * P : (d + 1) * P],
                        rhs=a_shared[:, fs, :],
                        start=(fs == 0),
                        stop=(fs == FST - 1),
                    )
            for d in range(DT):
                yo = ytmp.tile([P, TS], f32, tag="yo")
                nc.vector.tensor_scalar_add(yo[:], pys[d][:], bs2_sb[:, d : d + 1])
                nc.sync.dma_start(
                    ysT[:].rearrange("(o p) c -> p o c", p=P)[:, d, :], yo[:]
                )

            # ---------- expert-path inputs (prefetch during shared stage) ----------
            b1_sb = consts.tile([P, FT], f32)
            nc.sync.dma_start(b1_sb[:], b1[:])
            b3_sb = consts.tile([P, FT], f32)
            nc.sync.dma_start(b3_sb[:], b3[:])
            b2_sb = consts.tile([P, DT], f32)
            nc.sync.dma_start(b2_sb[:], b2[:])
            xg_sb = xgp.tile([P, DT, C], sdt)
            nc.sync.dma_start(xg_sb[:], xgT[:])
            w1_sb = wres.tile([P, DT, F], sdt, tag="w1res")
            nc.sync.dma_start(w1_sb[:], w1[:])
            w3_sb = wres.tile([P, DT, F], sdt, tag="w3res")
            nc.sync.dma_start(w3_sb[:], w3[:])
            w2_sb = wres.tile([P, FT, D], sdt, tag="w2res")
            nc.sync.dma_start(w2_sb[:], w2[:])
            gw_sb = consts.tile([P, C], f32, tag="gw")
            nc.sync.dma_start(gw_sb[:], gw[:].to_broadcast([P, C]))

            # ---------- expert path ----------
            for n, (cs, cw) in enumerate(CH):
                a_n = a_expert(n)
                ncols = slice(cs, cs + cw)
                for f in range(FT):
                    ph1 = psp.tile([P, TS], f32, tag="ph1")
                    ph3 = psp.tile([P, TS], f32, tag="ph3")
                    fcols = slice(f * P, (f + 1) * P)
                    for d in range(DT):
                        nc.tensor.matmul(
                            ph1[:, :cw],
                            lhsT=w1_sb[:, d, fcols],
                            rhs=xg_sb[:, d, ncols],
                            start=(d == 0),
                            stop=(d == DT - 1),
                        )
                    for d in range(DT):
                        nc.tensor.matmul(
                            ph3[:, :cw],
                            lhsT=w3_sb[:, d, fcols],
                            rhs=xg_sb[:, d, ncols],
                            start=(d == 0),
                            stop=(d == DT - 1),
                        )
                    h1 = htmp.tile([P, TS], f32, tag="h1")
                    nc.vector.tensor_scalar_add(
                        h1[:, :cw], ph1[:, :cw], b1_sb[:, f : f + 1]
                    )
                    prod = htmp.tile([P, TS], f32, tag="prod")
                    nc.vector.scalar_tensor_tensor(
                        prod[:, :cw],
                        in0=ph3[:, :cw],
                        scalar=b3_sb[:, f : f + 1],
                        in1=h1[:, :cw],
                        op0=mybir.AluOpType.add,
                        op1=mybir.AluOpType.mult,
                    )
                    nc.scalar.activation(
                        a_n[:, f, :cw], prod[:, :cw], mybir.ActivationFunctionType.Silu
                    )
                for dp in range(DT // 2):
                    dA, dB = 2 * dp, 2 * dp + 1
                    pyA = psp.tile([P, TS], f32, tag="ph1")
                    pyB = psp.tile([P, TS], f32, tag="ph3")
                    for f in range(FT):
                        nc.tensor.matmul(
                            pyA[:, :cw],
                            lhsT=w2_sb[:, f, dA * P : (dA + 1) * P],
                            rhs=a_n[:, f, :cw],
                            start=(f == 0),
                            stop=(f == FT - 1),
                        )
                        nc.tensor.matmul(
                            pyB[:, :cw],
                            lhsT=w2_sb[:, f, dB * P : (dB + 1) * P],
                            rhs=a_n[:, f, :cw],
                            start=(f == 0),
                            stop=(f == FT - 1),
                        )
                    for d, py in ((dA, pyA), (dB, pyB)):
                        yo = ytmp.tile([P, TS], f32, tag="yo")
                        nc.vector.scalar_tensor_tensor(
                            yo[:, :cw],
                            in0=py[:, :cw],
                            scalar=b2_sb[:, d : d + 1],
                            in1=gw_sb[:, ncols],
                            op0=mybir.AluOpType.add,
                            op1=mybir.AluOpType.mult,
                        )
                        nc.sync.dma_start(
                            yT[:].rearrange("(o p) c -> p o c", p=P)[:, d, ncols],
                            yo[:, :cw],
                        )

    nc.compile()
    return nc


def _get_program(C, cfg):
    key = (C, cfg)
    if key not in _COMPILED:
        _COMPILED[key] = build_program(C, cfg)
    return _COMPILED[key]


def _pack_bias(b):
    """[K] -> [128, K/128] partition-major (element (p, o) = b[o*128+p])."""
    b = np.asarray(b, dtype=np.float32)
    return np.ascontiguousarray(b.reshape(-1, P).T)


def _route(xf, Wg):
    """Host gating: softmax -> top-2 -> renormalized weights (float64)."""
    logits = xf.astype(np.float64) @ Wg.astype(np.float64)
    m = logits.max(-1, keepdims=True)
    p = np.exp(logits - m)
    scores = p / p.sum(-1, keepdims=True)
    eidx = np.argsort(-scores, axis=-1, kind="stable")[:, :TOPK]
    sel = np.take_along_axis(scores, eidx, -1)
    sm = sel.max(-1, keepdims=True)
    pe = np.exp(sel - sm)
    ew = pe / pe.sum(-1, keepdims=True)
    return eidx, ew.astype(np.float32)


def prepare_in_maps(x, Wg, W1, b1, W3, b3, W2, b2, Ws1, bs1, Ws3, bs3, Ws2, bs2, cfg=MM_CFG):
    xf = np.ascontiguousarray(np.asarray(x, dtype=np.float32).reshape(-1, D))
    eidx, ew = _route(xf, np.asarray(Wg, dtype=np.float32))

    flat_e = eidx.reshape(-1)
    flat_w = ew.reshape(-1)
    tok = np.repeat(np.arange(T), TOPK)
    order = np.argsort(flat_e, kind="stable")
    se, st, sw = flat_e[order], tok[order], flat_w[order]
    counts = np.bincount(se, minlength=E)
    offs = np.concatenate([[0], np.cumsum(counts)])
    idx_lists = [st[offs[e] : offs[e + 1]] for e in range(E)]
    gw_lists = [sw[offs[e] : offs[e + 1]] for e in range(E)]

    C = max(512, int(np.ceil(counts.max() / 8)) * 8)

    np_mdt = _np_mm_dtype(cfg)
    WG = 2 * P

    def tile_kxn(a, K):
        # [K, N] -> [P, K/P, N] partition-major
        a = np.asarray(a, dtype=np.float32)
        return a.reshape(K // P, P, -1).transpose(1, 0, 2).astype(np_mdt)

    # shared-expert weights are identical on every core: pack once
    ws1_t = np.asarray(Ws1, dtype=np.float32).reshape(D // P, P, FS // WG, WG)
    ws1_t = ws1_t.transpose(2, 1, 0, 3).astype(np_mdt)
    ws3_t = np.asarray(Ws3, dtype=np.float32).reshape(D // P, P, FS // WG, WG)
    ws3_t = ws3_t.transpose(2, 1, 0, 3).astype(np_mdt)
    ws2_t = np.asarray(Ws2, dtype=np.float32).astype(np_mdt)
    bs1_p, bs3_p, bs2_p = _pack_bias(bs1), _pack_bias(bs3), _pack_bias(bs2)

    in_maps = []
    for e in range(E):
        cnt = counts[e]
        xg = np.zeros((C, D), dtype=np.float32)
        xg[:cnt] = xf[idx_lists[e]]
        gwv = np.zeros((1, C), dtype=np.float32)
        gwv[0, :cnt] = gw_lists[e]
        xsl = xf[e * TS : (e + 1) * TS]
        in_maps.append(
            {
                "xgT": tile_kxn(xg.T, D),
                "gw": gwv,
                "w1": tile_kxn(W1[e], D),
                "w3": tile_kxn(W3[e], D),
                "w2": tile_kxn(W2[e], F),
                "b1": _pack_bias(b1[e]),
                "b3": _pack_bias(b3[e]),
                "b2": _pack_bias(b2[e]),
                "xsT": tile_kxn(xsl.T, D),
                "ws1": ws1_t,
                "ws3": ws3_t,
                "ws2": ws2_t,
                "bs1": bs1_p,
                "bs3": bs3_p,
                "bs2": bs2_p,
            }
        )
    return in_maps, idx_lists, counts, C


def combine(results, idx_lists, counts, x_shape, x_dtype):
    y = np.empty((D, T), dtype=np.float32)
    for e in range(E):
        y[:, e * TS : (e + 1) * TS] = results[e]["ysT"]
    for e in range(E):
        cnt = counts[e]
        if cnt:
            cols = idx_lists[e][:cnt]
            y[:, cols] += results[e]["yT"][:, :cnt]
    return np.ascontiguousarray(y.T).reshape(x_shape).astype(x_dtype, copy=False)


def run(x, Wg, W1, b1, W3, b3, W2, b2, Ws1, bs1, Ws3, bs3, Ws2, bs2,
        cfg=MM_CFG, trace=False, trace_kwargs=None):
    in_maps, idx_lists, counts, C = prepare_in_maps(
        x, Wg, W1, b1, W3, b3, W2, b2, Ws1, bs1, Ws3, bs3, Ws2, bs2, cfg
    )
    nc = _get_program(C, cfg)
    res = run_bass_kernel_spmd(
        nc, in_maps, list(range(E)), trace=trace, **(trace_kwargs or {})
    )
    out = combine(res.results, idx_lists, counts, np.asarray(x).shape, np.asarray(x).dtype)
    return out, res


def kernel(**inputs):
    out, _ = run(**inputs)
    return out
